# revision 29
# baseline (speedup 1.0000x reference)
"""Trainium2 Bass kernel for nn_AddSLoss (retrieval_knn).

Computes, per batch b:
  tf = model_points @ H[:3,:3].T + H[:3,3]
  d2[q,r] = ||tf_q - target_r||^2
  dis_sym[b] = mean_q sqrt(min_r d2[q,r])        (argmin+gather+norm == sqrt of min)
  dis_id[b]  = mean_q ||tf_q - target_q||
  dis[b] = dis_sym[b] if idx[b] in {0,1,2,3} else dis_id[b]

Sharding: data-parallel over batch; 16 batches -> 8 cores x 2 batches.

Device algorithm (per core, per batch):
  - load points in "fat" layout [128, 16, 3]  (point q = 16*p + j)
  - compute tf and augmented bf16 hi/lo split vectors in fat layout
  - u13 = [Ah,Ah,Al,Ph,Pl,1,1], v13 = [Bh,Bl,Bh,-1,-1,-Rh,-Rl]
    where A=tf, B=2*target, P=||tf||^2, R=||target||^2, h/l = bf16 hi/lo split.
    Then u13 . v13 = -d2 (to ~1e-4 abs), K=13 bf16 matmul at 1 cyc/row.
  - PE transposes build u13T/v13T [13, 2048] from the fat tiles
  - main: per q-chunk, 4 matmuls [128,512] -> PSUM holds -d2
  - reduce: ACT copies PSUM -> SBUF fp16 (1 elem/lane/cyc), DVE folds pairwise
    max at 2x into a per-batch collector [128, 16, 256]; one batched 1x
    reduce per batch finishes the per-q max(-d2) = -min d2.
  - finals: sqrt via ACT (scale=-1) with free-dim accum, partition sum via
    f32 PE transpose + DVE reduce, select sym/id by idx, scale 1/N
"""

import numpy as np

BS, N, NCORES = 16, 2048, 8
BPC = BS // NCORES          # batches per core = 2
NJ = N // 128               # 16 j-chunks (q-chunks)
KAUG = 13
KPAD = 32

_cache = {}

def _eye_bf16():
    try:
        import ml_dtypes
        return np.eye(128, dtype=ml_dtypes.bfloat16)
    except ImportError:  # bf16 bit pattern = top 16 bits of f32
        e = np.eye(128, dtype=np.float32)
        return (e.view(np.uint32) >> 16).astype(np.uint16)


_IDENT_BF16 = _eye_bf16()
_IDENT_F32 = np.eye(128, dtype=np.float32)


def _build():
    import concourse.bacc as bacc
    import concourse.mybir as mybir
    from concourse import tile
    from concourse import masks

    f32 = mybir.dt.float32
    bf16 = mybir.dt.bfloat16
    fp16 = mybir.dt.float16
    i32 = mybir.dt.int32
    MUL = mybir.AluOpType.mult
    ADD = mybir.AluOpType.add
    SUB = mybir.AluOpType.subtract
    MAX = mybir.AluOpType.max
    AX = mybir.AxisListType.X
    ACTF = mybir.ActivationFunctionType

    nc = bacc.Bacc("TRN2", target_bir_lowering=False, debug=False,
                   num_devices=NCORES)
    idb_d = nc.dram_tensor("ident_bf16", [128, 128], bf16, kind="ExternalInput")
    idf_d = nc.dram_tensor("ident_f32", [128, 128], f32, kind="ExternalInput")
    tgt_d = nc.dram_tensor("target", [BPC, N, 3], f32, kind="ExternalInput")
    mp_d = nc.dram_tensor("model_points", [BPC, N, 3], f32, kind="ExternalInput")
    h_d = nc.dram_tensor("H", [BPC, 4, 4], f32, kind="ExternalInput")
    idx_d = nc.dram_tensor("idx", [BPC, 1], i32, kind="ExternalInput")
    out_d = nc.dram_tensor("out", [1, BPC], f32, kind="ExternalOutput")

    with tile.TileContext(nc) as tc:
        with tc.tile_pool(name="const", bufs=1) as constp, \
             tc.tile_pool(name="sb", bufs=1) as sb, \
             tc.tile_pool(name="collp", bufs=1) as collp, \
             tc.tile_pool(name="fin", bufs=1) as fin:
            ident = constp.tile([128, 128], bf16)
            identf = constp.tile([128, 128], f32)
            nc.scalar.dma_start(ident[:], idb_d[:])
            nc.scalar.dma_start(identf[:], idf_d[:])
            ones16 = constp.tile([128, NJ], f32)
            nc.vector.memset(ones16[:], 1.0)
            negones16 = constp.tile([128, NJ], f32)
            nc.vector.memset(negones16[:], -1.0)

            out_sb = fin.tile([1, BPC], f32)

            uT, vT = [], []
            tf_all, tgt_all = [], []
            coll_all = []

            # all input loads issued up front on separate queues
            mp_t, tg_t, hb_t = [], [], []
            for b in range(BPC):
                mp = sb.tile([128, NJ, 3], f32, tag=f"mp{b}", name=f"mp{b}")
                tg = sb.tile([128, NJ, 3], f32, tag=f"tg{b}", name=f"tg{b}")
                hb = sb.tile([128, 16], f32, tag=f"hb{b}", name=f"hb{b}")
                mp_t.append(mp); tg_t.append(tg); hb_t.append(hb)
            # H rows into partitions 0/32/64/96 first (tiny), then big loads
            hr4 = []
            for b in range(BPC):
                h4 = sb.tile([128, 16], f32, tag=f"h4{b}", name=f"h4{b}")
                nc.vector.memset(h4[:], 0.0)
                eng = nc.sync if b == 0 else nc.scalar
                for g in range(4):
                    eng.dma_start(h4[32 * g:32 * g + 1, :],
                                  h_d.rearrange("b x y -> b (x y)")[b:b + 1])
                hr4.append(h4)
            nc.sync.dma_start(mp_t[0][:], mp_d[0].rearrange("(p j) c -> p j c", p=128))
            nc.gpsimd.dma_start(tg_t[0][:], tgt_d[0].rearrange("(p j) c -> p j c", p=128))
            nc.scalar.dma_start(mp_t[1][:], mp_d[1].rearrange("(p j) c -> p j c", p=128))
            nc.sync.dma_start(tg_t[1][:], tgt_d[1].rearrange("(p j) c -> p j c", p=128))
            nc.vector.stream_shuffle(hb_t[0][:], hr4[0][:], mask=[0] * 32)
            nc.vector.stream_shuffle(hb_t[1][:], hr4[1][:], mask=[0] * 32)

            # idx -> sym mask [1, BPC] (after big loads on gpsimd queue)
            idxi = constp.tile([1, BPC], i32)
            nc.gpsimd.dma_start(idxi[:], idx_d.rearrange("b one -> one b"))
            idxf = constp.tile([1, BPC], f32)
            nc.vector.tensor_copy(idxf[:], idxi[:])
            symf = constp.tile([1, BPC], f32)
            nc.vector.tensor_scalar(symf[:], idxf[:], 3.5, None,
                                    op0=mybir.AluOpType.is_lt)

            d2_cm = tc.tile_pool(name="d2", bufs=2, space="PSUM")
            d2p = d2_cm.__enter__()

            for b in range(BPC):
                # ---------- prep (fat layout) ----------
                mp, tg, hb = mp_t[b], tg_t[b], hb_t[b]

                # V-side prep first (independent of H broadcast)
                b2 = sb.tile([128, NJ, 3], f32, tag=f"b2{b}")  # 2*target
                nc.scalar.mul(b2[:], tg[:], 2.0)
                sqt = sb.tile([128, NJ, 3], f32, tag=f"sqt{b}")
                nntg = sb.tile([128, NJ], f32, tag=f"nntg{b}")  # -||t||^2
                nc.scalar.square(sqt[:], tg[:])
                nc.vector.tensor_reduce(nntg[:], sqt[:], op=ADD, axis=AX,
                                        negate=True)
                tgt_all.append(tg)

                # V bf16 build early (only needs target-side data)
                V = sb.tile([128, NJ, KAUG], bf16, tag=f"V{b}")
                nc.vector.tensor_copy(V[:, :, 0:3], b2[:])
                nc.vector.tensor_tensor(V[:, :, 3:6], b2[:], V[:, :, 0:3], op=SUB)
                nc.vector.tensor_copy(V[:, :, 6:9], V[:, :, 0:3])
                nc.vector.tensor_copy(V[:, :, 11], nntg[:])
                nc.vector.tensor_tensor(V[:, :, 12], nntg[:], V[:, :, 11], op=SUB)
                nc.vector.tensor_copy(V[:, :, 9], negones16[:])
                nc.vector.tensor_copy(V[:, :, 10], negones16[:])

                # tf[p,j,e] = sum_d mp[p,j,d]*H[e,d] + H[e,3]   (DVE)
                tf = sb.tile([128, NJ, 3], f32, tag=f"tf{b}")
                tmp1 = sb.tile([128, NJ], f32, tag=f"tmp1{b}")
                tmp2 = sb.tile([128, NJ], f32, tag=f"tmp2{b}")
                for e in range(3):
                    nc.vector.tensor_scalar(tmp1[:], mp[:, :, 0],
                                            hb[:, 4 * e:4 * e + 1],
                                            hb[:, 4 * e + 3:4 * e + 4],
                                            op0=MUL, op1=ADD)
                    nc.vector.scalar_tensor_tensor(tmp2[:], mp[:, :, 1],
                                                   hb[:, 4 * e + 1:4 * e + 2],
                                                   tmp1[:], op0=MUL, op1=ADD)
                    nc.vector.scalar_tensor_tensor(tf[:, :, e], mp[:, :, 2],
                                                   hb[:, 4 * e + 2:4 * e + 3],
                                                   tmp2[:], op0=MUL, op1=ADD)
                tf_all.append(tf)

                # norms (squares on ACT, reduces on DVE)
                sq = sb.tile([128, NJ, 3], f32, tag=f"sq{b}")
                ntf = sb.tile([128, NJ], f32, tag=f"ntf{b}")
                nc.scalar.square(sq[:], tf[:])
                nc.vector.tensor_reduce(ntf[:], sq[:], op=ADD, axis=AX)

                # U bf16 build
                U = sb.tile([128, NJ, KAUG], bf16, tag=f"U{b}")
                # U rows: 0:3 Ah, 3:6 Ah, 6:9 Al, 9 Ph, 10 Pl, 11 one, 12 one
                nc.vector.tensor_copy(U[:, :, 0:3], tf[:])
                nc.vector.tensor_tensor(U[:, :, 6:9], tf[:], U[:, :, 0:3], op=SUB)
                nc.vector.tensor_copy(U[:, :, 3:6], U[:, :, 0:3])
                nc.vector.tensor_copy(U[:, :, 9], ntf[:])
                nc.vector.tensor_tensor(U[:, :, 10], ntf[:], U[:, :, 9], op=SUB)
                nc.vector.tensor_copy(U[:, :, 11], ones16[:])
                nc.vector.tensor_copy(U[:, :, 12], ones16[:])

                # ---------- transposes to [13, 2048] via shared d2 psum slots ----------
                uTb = sb.tile([KAUG, N], bf16, tag=f"uT{b}")
                vTb = sb.tile([KAUG, N], bf16, tag=f"vT{b}")
                for (fat, Tsb) in ((V, vTb), (U, uTb)):
                    tps = d2p.tile([128, N], f32, tag="d2")
                    tpsb = tps[0:KAUG, :].bitcast(bf16)  # [13, 4096] bf16 view
                    for j in range(NJ):
                        nc.tensor.transpose(
                            tpsb[:, 128 * j:128 * (j + 1)],
                            fat[:, j, :], ident[:])
                    for g in range(4):
                        nc.vector.tensor_copy(
                            Tsb[:, 512 * g:512 * (g + 1)],
                            tpsb[:, 512 * g:512 * (g + 1)])
                uT.append(uTb)
                vT.append(vTb)
                coll = collp.tile([128, NJ, 256], fp16, tag=f"coll{b}")
                coll_all.append(coll)

            # ---------- main: matmuls + reduce ----------
            accs_all, acci_all = [], []
            with tc.tile_pool(name="fold", bufs=6) as foldp:
                for b in range(BPC):
                    negmin = fin.tile([128, NJ], f32, tag=f"nm{b}")
                    for qc in range(NJ):
                        lhsT = uT[b][:, 128 * qc:128 * (qc + 1)]
                        ps = d2p.tile([128, N], f32, tag="d2")
                        for k in range(4):
                            nc.tensor.matmul(ps[:, 512 * k:512 * (k + 1)], lhsT,
                                             vT[b][:, 512 * k:512 * (k + 1)])
                        # ACT drains PSUM to SBUF fp16; DVE folds at 2x
                        cp = foldp.tile([128, N], fp16, tag="cp")
                        nc.scalar.copy(cp[:], ps[:])
                        f1 = foldp.tile([128, 2, 512], fp16, tag="f1")
                        nc.vector.tensor_tensor(f1[:, 0, :], cp[:, 0:512],
                                                cp[:, 512:1024], op=MAX)
                        nc.vector.tensor_tensor(f1[:, 1, :], cp[:, 1024:1536],
                                                cp[:, 1536:2048], op=MAX)
                        f2 = foldp.tile([128, 512], fp16, tag="f2")
                        nc.vector.tensor_tensor(f2[:], f1[:, 0, :],
                                                f1[:, 1, :], op=MAX)
                        nc.vector.tensor_tensor(coll_all[b][:, qc, :],
                                                f2[:, 0:256], f2[:, 256:512],
                                                op=MAX)
                        if qc == NJ // 2 - 1:
                            nc.vector.tensor_reduce(
                                negmin[:, 0:NJ // 2],
                                coll_all[b][:, 0:NJ // 2, :], op=MAX, axis=AX)
                        if qc == 3 * NJ // 4 - 1:
                            nc.vector.tensor_reduce(
                                negmin[:, NJ // 2:3 * NJ // 4],
                                coll_all[b][:, NJ // 2:3 * NJ // 4, :],
                                op=MAX, axis=AX)
                        if qc == NJ - 2:
                            nc.vector.tensor_reduce(
                                negmin[:, 3 * NJ // 4:NJ - 1],
                                coll_all[b][:, 3 * NJ // 4:NJ - 1, :],
                                op=MAX, axis=AX)
                    nc.vector.tensor_reduce(
                        negmin[:, NJ - 1:NJ],
                        coll_all[b][:, NJ - 1:NJ, :], op=MAX, axis=AX)
                    # heavy finals overlap the other batch's main pipeline
                    rt = fin.tile([128, NJ], f32, tag=f"rt{b}")
                    accs = fin.tile([128, 1], f32, tag=f"as{b}")
                    nc.scalar.activation(rt[:], negmin[:], ACTF.Sqrt,
                                         scale=-1.0, accum_out=accs[:])
                    diff = fin.tile([128, NJ, 3], f32, tag=f"df{b}")
                    nc.vector.tensor_tensor(diff[:], tf_all[b][:], tgt_all[b][:],
                                            op=SUB)
                    dsq = fin.tile([128, NJ, 3], f32, tag=f"dq{b}")
                    nc.scalar.square(dsq[:], diff[:])
                    nid = fin.tile([128, NJ], f32, tag=f"ni{b}")
                    nc.vector.tensor_reduce(nid[:], dsq[:], op=ADD, axis=AX)
                    rti = fin.tile([128, NJ], f32, tag=f"ri{b}")
                    acci = fin.tile([128, 1], f32, tag=f"ai{b}")
                    nc.scalar.activation(rti[:], nid[:], ACTF.Sqrt,
                                         accum_out=acci[:])
                    accs_all.append(accs)
                    acci_all.append(acci)

            # ---------- tail: partition sums + select (reuse d2 slots) ----------
            if True:
                for b in range(BPC):
                    fts = d2p.tile([128, N], f32, tag="d2")
                    tps_s = fts[0:1, 0:128]
                    nc.tensor.transpose(tps_s, accs_all[b][:], identf[:])
                    tps_i = fts[0:1, 512:640]
                    nc.tensor.transpose(tps_i, acci_all[b][:], identf[:])
                    s_sym = fin.tile([1, 1], f32, tag=f"ss{b}")
                    s_id = fin.tile([1, 1], f32, tag=f"si{b}")
                    nc.vector.tensor_reduce(s_sym[:], tps_s[:], op=ADD, axis=AX)
                    nc.vector.tensor_reduce(s_id[:], tps_i[:], op=ADD, axis=AX)
                    dd = fin.tile([1, 1], f32, tag=f"dd{b}")
                    nc.vector.tensor_tensor(dd[:], s_sym[:], s_id[:], op=SUB)
                    dd2 = fin.tile([1, 1], f32, tag=f"dd2{b}")
                    nc.vector.scalar_tensor_tensor(dd2[:], dd[:],
                                                   symf[:, b:b + 1],
                                                   s_id[:],
                                                   op0=MUL, op1=ADD)
                    nc.vector.tensor_scalar_mul(out_sb[:, b:b + 1], dd2[:], 1.0 / N)
            d2_cm.__exit__(None, None, None)
            nc.sync.dma_start(out_d[:], out_sb[:])
    nc.compile()
    return nc


def _get_nc():
    if "nc" not in _cache:
        _cache["nc"] = _build()
    return _cache["nc"]


def kernel(target, model_points, idx, H):
    from concourse.bass_utils import run_bass_kernel_spmd

    nc = _get_nc()
    target = np.ascontiguousarray(np.asarray(target, dtype=np.float32))
    model_points = np.ascontiguousarray(np.asarray(model_points, dtype=np.float32))
    H = np.ascontiguousarray(np.asarray(H, dtype=np.float32))
    idx_i = np.ascontiguousarray(np.asarray(idx).astype(np.int32))

    in_maps = []
    for c in range(NCORES):
        s = slice(c * BPC, (c + 1) * BPC)
        in_maps.append({
            "target": target[s],
            "model_points": model_points[s],
            "H": H[s],
            "idx": idx_i[s],
            "ident_bf16": _IDENT_BF16,
            "ident_f32": _IDENT_F32,
        })
    res = run_bass_kernel_spmd(nc, in_maps, list(range(NCORES)))
    out = np.concatenate([r["out"].reshape(-1) for r in res.results])
    return out.astype(np.float32)


# revision 30
# speedup vs baseline: 1.0083x; 1.0083x over previous
"""Trainium2 Bass kernel for nn_AddSLoss (retrieval_knn).

Computes, per batch b:
  tf = model_points @ H[:3,:3].T + H[:3,3]
  d2[q,r] = ||tf_q - target_r||^2
  dis_sym[b] = mean_q sqrt(min_r d2[q,r])        (argmin+gather+norm == sqrt of min)
  dis_id[b]  = mean_q ||tf_q - target_q||
  dis[b] = dis_sym[b] if idx[b] in {0,1,2,3} else dis_id[b]

Sharding: data-parallel over batch; 16 batches -> 8 cores x 2 batches.

Device algorithm (per core, per batch):
  - load points in "fat" layout [128, 16, 3]  (point q = 16*p + j)
  - compute tf and augmented bf16 hi/lo split vectors in fat layout
  - u13 = [Ah,Ah,Al,Ph,Pl,1,1], v13 = [Bh,Bl,Bh,-1,-1,-Rh,-Rl]
    where A=tf, B=2*target, P=||tf||^2, R=||target||^2, h/l = bf16 hi/lo split.
    Then u13 . v13 = -d2 (to ~1e-4 abs), K=13 bf16 matmul at 1 cyc/row.
  - PE transposes build u13T/v13T [13, 2048] from the fat tiles
  - main: per q-chunk, 4 matmuls [128,512] -> PSUM holds -d2
  - reduce: ACT copies PSUM -> SBUF fp16 (1 elem/lane/cyc), DVE folds pairwise
    max at 2x into a per-batch collector [128, 16, 256]; one batched 1x
    reduce per batch finishes the per-q max(-d2) = -min d2.
  - finals: sqrt via ACT (scale=-1) with free-dim accum, partition sum via
    f32 PE transpose + DVE reduce, select sym/id by idx, scale 1/N
"""

import numpy as np

BS, N, NCORES = 16, 2048, 8
BPC = BS // NCORES          # batches per core = 2
NJ = N // 128               # 16 j-chunks (q-chunks)
KAUG = 13
KPAD = 32

_cache = {}

def _eye_bf16():
    try:
        import ml_dtypes
        return np.eye(128, dtype=ml_dtypes.bfloat16)
    except ImportError:  # bf16 bit pattern = top 16 bits of f32
        e = np.eye(128, dtype=np.float32)
        return (e.view(np.uint32) >> 16).astype(np.uint16)


_IDENT_BF16 = _eye_bf16()
_IDENT_F32 = np.eye(128, dtype=np.float32)


def _build():
    import concourse.bacc as bacc
    import concourse.mybir as mybir
    from concourse import tile
    from concourse import masks

    f32 = mybir.dt.float32
    bf16 = mybir.dt.bfloat16
    fp16 = mybir.dt.float16
    i32 = mybir.dt.int32
    MUL = mybir.AluOpType.mult
    ADD = mybir.AluOpType.add
    SUB = mybir.AluOpType.subtract
    MAX = mybir.AluOpType.max
    AX = mybir.AxisListType.X
    ACTF = mybir.ActivationFunctionType

    nc = bacc.Bacc("TRN2", target_bir_lowering=False, debug=False,
                   num_devices=NCORES)
    idb_d = nc.dram_tensor("ident_bf16", [128, 128], bf16, kind="ExternalInput")
    idf_d = nc.dram_tensor("ident_f32", [128, 128], f32, kind="ExternalInput")
    tgt_d = nc.dram_tensor("target", [BPC, N, 3], f32, kind="ExternalInput")
    mp_d = nc.dram_tensor("model_points", [BPC, N, 3], f32, kind="ExternalInput")
    h_d = nc.dram_tensor("H", [BPC, 4, 4], f32, kind="ExternalInput")
    idx_d = nc.dram_tensor("idx", [BPC, 1], i32, kind="ExternalInput")
    out_d = nc.dram_tensor("out", [1, BPC], f32, kind="ExternalOutput")

    with tile.TileContext(nc) as tc:
        with tc.tile_pool(name="const", bufs=1) as constp, \
             tc.tile_pool(name="sb", bufs=1) as sb, \
             tc.tile_pool(name="collp", bufs=1) as collp, \
             tc.tile_pool(name="fin", bufs=1) as fin:
            ident = constp.tile([128, 128], bf16)
            identf = constp.tile([128, 128], f32)
            nc.scalar.dma_start(ident[:], idb_d[:])
            nc.scalar.dma_start(identf[:], idf_d[:])
            ones16 = constp.tile([128, NJ], f32)
            nc.vector.memset(ones16[:], 1.0)
            negones16 = constp.tile([128, NJ], f32)
            nc.vector.memset(negones16[:], -1.0)

            out_sb = fin.tile([1, BPC], f32)

            uT, vT = [], []
            tf_all, tgt_all = [], []
            coll_all = []

            # all input loads issued up front on separate queues
            mp_t, tg_t, hb_t = [], [], []
            for b in range(BPC):
                mp = sb.tile([128, NJ, 3], f32, tag=f"mp{b}", name=f"mp{b}")
                tg = sb.tile([128, NJ, 3], f32, tag=f"tg{b}", name=f"tg{b}")
                hb = sb.tile([128, 16], f32, tag=f"hb{b}", name=f"hb{b}")
                mp_t.append(mp); tg_t.append(tg); hb_t.append(hb)
            # H rows into partitions 0/32/64/96 first (tiny), then big loads
            hr4 = []
            for b in range(BPC):
                h4 = sb.tile([128, 16], f32, tag=f"h4{b}", name=f"h4{b}")
                nc.vector.memset(h4[:], 0.0)
                eng = nc.sync if b == 0 else nc.scalar
                for g in range(4):
                    eng.dma_start(h4[32 * g:32 * g + 1, :],
                                  h_d.rearrange("b x y -> b (x y)")[b:b + 1])
                hr4.append(h4)
            nc.sync.dma_start(mp_t[0][:], mp_d[0].rearrange("(p j) c -> p j c", p=128))
            nc.gpsimd.dma_start(tg_t[0][:], tgt_d[0].rearrange("(p j) c -> p j c", p=128))
            nc.scalar.dma_start(mp_t[1][:], mp_d[1].rearrange("(p j) c -> p j c", p=128))
            nc.sync.dma_start(tg_t[1][:], tgt_d[1].rearrange("(p j) c -> p j c", p=128))
            nc.vector.stream_shuffle(hb_t[0][:], hr4[0][:], mask=[0] * 32)
            nc.vector.stream_shuffle(hb_t[1][:], hr4[1][:], mask=[0] * 32)

            # idx -> sym mask [1, BPC] (after big loads on gpsimd queue)
            idxi = constp.tile([1, BPC], i32)
            nc.gpsimd.dma_start(idxi[:], idx_d.rearrange("b one -> one b"))
            idxf = constp.tile([1, BPC], f32)
            nc.vector.tensor_copy(idxf[:], idxi[:])
            symf = constp.tile([1, BPC], f32)
            nc.vector.tensor_scalar(symf[:], idxf[:], 3.5, None,
                                    op0=mybir.AluOpType.is_lt)

            d2_cm = tc.tile_pool(name="d2", bufs=2, space="PSUM")
            d2p = d2_cm.__enter__()

            for b in range(BPC):
                # ---------- prep (fat layout) ----------
                mp, tg, hb = mp_t[b], tg_t[b], hb_t[b]

                # V-side prep first (independent of H broadcast)
                b2 = sb.tile([128, NJ, 3], f32, tag=f"b2{b}")  # 2*target
                nc.scalar.mul(b2[:], tg[:], 2.0)
                sqt = sb.tile([128, NJ, 3], f32, tag=f"sqt{b}")
                nntg = sb.tile([128, NJ], f32, tag=f"nntg{b}")  # -||t||^2
                nc.scalar.square(sqt[:], tg[:])
                nc.vector.tensor_reduce(nntg[:], sqt[:], op=ADD, axis=AX,
                                        negate=True)
                tgt_all.append(tg)

                # V bf16 build early (only needs target-side data)
                V = sb.tile([128, NJ, KAUG], bf16, tag=f"V{b}")
                nc.vector.tensor_copy(V[:, :, 0:3], b2[:])
                nc.vector.tensor_tensor(V[:, :, 3:6], b2[:], V[:, :, 0:3], op=SUB)
                nc.vector.tensor_copy(V[:, :, 6:9], V[:, :, 0:3])
                nc.vector.tensor_copy(V[:, :, 11], nntg[:])
                nc.vector.tensor_tensor(V[:, :, 12], nntg[:], V[:, :, 11], op=SUB)
                nc.vector.tensor_copy(V[:, :, 9], negones16[:])
                nc.vector.tensor_copy(V[:, :, 10], negones16[:])

                # tf[p,j,e] = sum_d mp[p,j,d]*H[e,d] + H[e,3]   (DVE)
                tf = sb.tile([128, NJ, 3], f32, tag=f"tf{b}")
                tmp1 = sb.tile([128, NJ], f32, tag=f"tmp1{b}")
                tmp2 = sb.tile([128, NJ], f32, tag=f"tmp2{b}")
                for e in range(3):
                    nc.vector.tensor_scalar(tmp1[:], mp[:, :, 0],
                                            hb[:, 4 * e:4 * e + 1],
                                            hb[:, 4 * e + 3:4 * e + 4],
                                            op0=MUL, op1=ADD)
                    nc.vector.scalar_tensor_tensor(tmp2[:], mp[:, :, 1],
                                                   hb[:, 4 * e + 1:4 * e + 2],
                                                   tmp1[:], op0=MUL, op1=ADD)
                    nc.vector.scalar_tensor_tensor(tf[:, :, e], mp[:, :, 2],
                                                   hb[:, 4 * e + 2:4 * e + 3],
                                                   tmp2[:], op0=MUL, op1=ADD)
                tf_all.append(tf)

                # norms (squares on ACT, reduces on DVE)
                sq = sb.tile([128, NJ, 3], f32, tag=f"sq{b}")
                ntf = sb.tile([128, NJ], f32, tag=f"ntf{b}")
                nc.scalar.square(sq[:], tf[:])
                nc.vector.tensor_reduce(ntf[:], sq[:], op=ADD, axis=AX)

                # U bf16 build
                U = sb.tile([128, NJ, KAUG], bf16, tag=f"U{b}")
                # U rows: 0:3 Ah, 3:6 Ah, 6:9 Al, 9 Ph, 10 Pl, 11 one, 12 one
                nc.vector.tensor_copy(U[:, :, 0:3], tf[:])
                nc.vector.tensor_tensor(U[:, :, 6:9], tf[:], U[:, :, 0:3], op=SUB)
                nc.vector.tensor_copy(U[:, :, 3:6], U[:, :, 0:3])
                nc.vector.tensor_copy(U[:, :, 9], ntf[:])
                nc.vector.tensor_tensor(U[:, :, 10], ntf[:], U[:, :, 9], op=SUB)
                nc.vector.tensor_copy(U[:, :, 11], ones16[:])
                nc.vector.tensor_copy(U[:, :, 12], ones16[:])

                # ---------- transposes to [13, 2048] via shared d2 psum slots ----------
                uTb = sb.tile([KAUG, N], bf16, tag=f"uT{b}")
                vTb = sb.tile([KAUG, N], bf16, tag=f"vT{b}")
                for (fat, Tsb) in ((V, vTb), (U, uTb)):
                    tps = d2p.tile([128, N], f32, tag="d2")
                    tpsb = tps[0:KAUG, :].bitcast(bf16)  # [13, 4096] bf16 view
                    for j in range(NJ):
                        nc.tensor.transpose(
                            tpsb[:, 128 * j:128 * (j + 1)],
                            fat[:, j, :], ident[:])
                    for g in range(4):
                        nc.vector.tensor_copy(
                            Tsb[:, 512 * g:512 * (g + 1)],
                            tpsb[:, 512 * g:512 * (g + 1)])
                uT.append(uTb)
                vT.append(vTb)
                coll = collp.tile([128, NJ, 256], fp16, tag=f"coll{b}")
                coll_all.append(coll)

            # ---------- main: matmuls + reduce ----------
            accs_all, acci_all = [], []
            with tc.tile_pool(name="fold", bufs=6) as foldp:
                for b in range(BPC):
                    negmin = fin.tile([128, NJ], f32, tag=f"nm{b}")
                    for qc in range(NJ):
                        lhsT = uT[b][:, 128 * qc:128 * (qc + 1)]
                        ps = d2p.tile([128, N], f32, tag="d2")
                        for k in range(4):
                            nc.tensor.matmul(ps[:, 512 * k:512 * (k + 1)], lhsT,
                                             vT[b][:, 512 * k:512 * (k + 1)])
                        # ACT drains PSUM to SBUF fp16; DVE folds at 2x
                        cp = foldp.tile([128, N], fp16, tag="cp")
                        nc.scalar.copy(cp[:], ps[:])
                        f1 = foldp.tile([128, 2, 512], fp16, tag="f1")
                        nc.vector.tensor_tensor(f1[:, 0, :], cp[:, 0:512],
                                                cp[:, 512:1024], op=MAX)
                        nc.vector.tensor_tensor(f1[:, 1, :], cp[:, 1024:1536],
                                                cp[:, 1536:2048], op=MAX)
                        f2 = foldp.tile([128, 512], fp16, tag="f2")
                        nc.vector.tensor_tensor(f2[:], f1[:, 0, :],
                                                f1[:, 1, :], op=MAX)
                        nc.vector.tensor_tensor(coll_all[b][:, qc, :],
                                                f2[:, 0:256], f2[:, 256:512],
                                                op=MAX)
                        if qc == NJ // 2 - 1:
                            nc.vector.tensor_reduce(
                                negmin[:, 0:NJ // 2],
                                coll_all[b][:, 0:NJ // 2, :], op=MAX, axis=AX)
                        if qc == 3 * NJ // 4 - 1:
                            nc.vector.tensor_reduce(
                                negmin[:, NJ // 2:3 * NJ // 4],
                                coll_all[b][:, NJ // 2:3 * NJ // 4, :],
                                op=MAX, axis=AX)
                        if qc == NJ - 2:
                            nc.vector.tensor_reduce(
                                negmin[:, 3 * NJ // 4:NJ - 1],
                                coll_all[b][:, 3 * NJ // 4:NJ - 1, :],
                                op=MAX, axis=AX)
                    nc.vector.tensor_reduce(
                        negmin[:, NJ - 1:NJ],
                        coll_all[b][:, NJ - 1:NJ, :], op=MAX, axis=AX)
                    # heavy finals overlap the other batch's main pipeline
                    rt = fin.tile([128, NJ], f32, tag=f"rt{b}")
                    accs = fin.tile([128, 1], f32, tag=f"as{b}")
                    nc.scalar.activation(rt[:], negmin[:], ACTF.Sqrt,
                                         scale=-1.0, accum_out=accs[:])
                    diff = fin.tile([128, NJ, 3], f32, tag=f"df{b}")
                    nc.vector.tensor_tensor(diff[:], tf_all[b][:], tgt_all[b][:],
                                            op=SUB)
                    dsq = fin.tile([128, NJ, 3], f32, tag=f"dq{b}")
                    nc.scalar.square(dsq[:], diff[:])
                    nid = fin.tile([128, NJ], f32, tag=f"ni{b}")
                    nc.vector.tensor_reduce(nid[:], dsq[:], op=ADD, axis=AX)
                    rti = fin.tile([128, NJ], f32, tag=f"ri{b}")
                    acci = fin.tile([128, 1], f32, tag=f"ai{b}")
                    nc.scalar.activation(rti[:], nid[:], ACTF.Sqrt,
                                         accum_out=acci[:])
                    accs_all.append(accs)
                    acci_all.append(acci)

            d2_cm.__exit__(None, None, None)

            # ---------- tail: partition sums + select ----------
            with tc.tile_pool(name="fpsum", bufs=2, space="PSUM") as fps:
                for b in range(BPC):
                    tps_s = fps.tile([1, 128], f32, tag="fts")
                    nc.tensor.transpose(tps_s[:], accs_all[b][:], identf[:])
                    tps_i = fps.tile([1, 128], f32, tag="fti")
                    nc.tensor.transpose(tps_i[:], acci_all[b][:], identf[:])
                    s_sym = fin.tile([1, 1], f32, tag=f"ss{b}")
                    s_id = fin.tile([1, 1], f32, tag=f"si{b}")
                    nc.vector.tensor_reduce(s_sym[:], tps_s[:], op=ADD, axis=AX)
                    nc.vector.tensor_reduce(s_id[:], tps_i[:], op=ADD, axis=AX)
                    dd = fin.tile([1, 1], f32, tag=f"dd{b}")
                    nc.vector.tensor_tensor(dd[:], s_sym[:], s_id[:], op=SUB)
                    dd2 = fin.tile([1, 1], f32, tag=f"dd2{b}")
                    nc.vector.scalar_tensor_tensor(dd2[:], dd[:],
                                                   symf[:, b:b + 1],
                                                   s_id[:],
                                                   op0=MUL, op1=ADD)
                    nc.vector.tensor_scalar_mul(out_sb[:, b:b + 1], dd2[:], 1.0 / N)
            nc.sync.dma_start(out_d[:], out_sb[:])
    nc.compile()
    return nc


def _get_nc():
    if "nc" not in _cache:
        _cache["nc"] = _build()
    return _cache["nc"]


def kernel(target, model_points, idx, H):
    from concourse.bass_utils import run_bass_kernel_spmd

    nc = _get_nc()
    target = np.ascontiguousarray(np.asarray(target, dtype=np.float32))
    model_points = np.ascontiguousarray(np.asarray(model_points, dtype=np.float32))
    H = np.ascontiguousarray(np.asarray(H, dtype=np.float32))
    idx_i = np.ascontiguousarray(np.asarray(idx).astype(np.int32))

    in_maps = []
    for c in range(NCORES):
        s = slice(c * BPC, (c + 1) * BPC)
        in_maps.append({
            "target": target[s],
            "model_points": model_points[s],
            "H": H[s],
            "idx": idx_i[s],
            "ident_bf16": _IDENT_BF16,
            "ident_f32": _IDENT_F32,
        })
    res = run_bass_kernel_spmd(nc, in_maps, list(range(NCORES)))
    out = np.concatenate([r["out"].reshape(-1) for r in res.results])
    return out.astype(np.float32)
